# revision 6
# baseline (speedup 1.0000x reference)
"""GNNPolicy bipartite-GNN kernel for 8 TRN2 NeuronCores (Bass/Tile).

See module notes at bottom. kernel(**inputs) -> np.ndarray [1].
Design summary:
 - Aggregation targets degree-sorted into 128-target tiles; tiles grouped in
   rounds of 8 (one per core, shared instruction stream, shared per-round
   slot cap).
 - conv v->c runs entirely from host-prepared per-edge-slot scalar streams
   (variable features are scalars); the embedding MLP runs per slot on device
   (feature-major, ACT-fused).
 - LN decomposes into per-slot scalars (mean/var quadratic in edge feature);
   alpha>0 commutes with relu and is applied as a per-partition scale during
   PSUM-accumulated aggregation.
 - Between launches the host re-shards device-computed per-constraint tables
   into conv c->v per-slot streams (index-space row movement only).
"""
import sys
from contextlib import ExitStack

import numpy as np
import ml_dtypes

if "/opt/trn_rl_repo" not in sys.path:
    sys.path.insert(0, "/opt/trn_rl_repo")

import concourse.bass as bass  # noqa: E402
import concourse.tile as tile  # noqa: E402
from concourse import bacc, mybir  # noqa: E402

f32 = mybir.dt.float32
bf16 = mybir.dt.bfloat16
bf = ml_dtypes.bfloat16

P = 128
NCORES = 8
EMB = 64
LN_EPS = 1e-5
SLAB = 512

Relu = mybir.ActivationFunctionType.Relu
Identity = mybir.ActivationFunctionType.Identity
Sqrt = mybir.ActivationFunctionType.Sqrt
Copy = mybir.ActivationFunctionType.Copy
ADD = mybir.AluOpType.add
SUB = mybir.AluOpType.subtract
MULT = mybir.AluOpType.mult


def ceil_div(a, b):
    return (a + b - 1) // b


def _slabs(n, width=SLAB):
    return [(i, min(width, n - i)) for i in range(0, n, width)]


# ---------------------------------------------------------------------------
# host planning
# ---------------------------------------------------------------------------
class ConvPlan:
    def __init__(self, tgt_idx, n_tgt):
        E = len(tgt_idx)
        deg = np.bincount(tgt_idx, minlength=n_tgt)
        order = np.argsort(-deg, kind="stable")
        n_rounds = ceil_div(n_tgt, NCORES * P)
        n_pad = n_rounds * NCORES * P
        self.n_tgt, self.n_rounds = n_tgt, n_rounds
        self.perm = np.concatenate([order, np.full(n_pad - n_tgt, -1, np.int64)])
        inv = np.empty(n_tgt, np.int64)
        inv[order] = np.arange(n_tgt)
        deg_sorted = np.concatenate([deg[order], np.zeros(n_pad - n_tgt, np.int64)])
        caps = deg_sorted.reshape(n_rounds, NCORES * P).max(axis=1)
        self.caps = np.maximum(caps, 1).astype(np.int64)
        self.total_slots = int(self.caps.sum()) * P
        self.round_off = np.concatenate([[0], np.cumsum(self.caps)[:-1]]) * P

        pos = inv[tgt_idx]
        rnd = pos // (NCORES * P)
        core = (pos // P) % NCORES
        part = pos % P
        sidx = np.argsort(pos, kind="stable")
        psort = pos[sidx]
        starts = np.r_[0, np.where(np.diff(psort) != 0)[0] + 1]
        ranks = np.arange(E) - np.repeat(starts, np.diff(np.r_[starts, E]))
        slot = np.empty(E, np.int64)
        slot[sidx] = ranks
        self.edge_core = core
        self.edge_pos = self.round_off[rnd] + slot * P + part

    def scatter(self, values, dtype=np.float32, fill=0.0):
        out = np.full((NCORES, self.total_slots), fill, dtype)
        out[self.edge_core, self.edge_pos] = values.astype(dtype)
        return out

    def target_rows(self, core):
        ids = np.empty(self.n_rounds * P, np.int64)
        for r in range(self.n_rounds):
            base = r * NCORES * P + core * P
            ids[r * P:(r + 1) * P] = self.perm[base:base + P]
        return ids


# ---------------------------------------------------------------------------
# constants packing
# ---------------------------------------------------------------------------
class ConstPack:
    def __init__(self, rows, dtype):
        self.rows, self.dtype = rows, dtype
        self.cols = []
        self.off = {}

    def add(self, name, arr):
        arr = np.asarray(arr, np.float32)
        if arr.ndim == 1:
            arr = arr[:, None]
        h, w = arr.shape
        blk = np.zeros((self.rows, w), np.float32)
        blk[:h] = arr
        self.off[name] = (sum(c.shape[1] for c in self.cols), w, h)
        self.cols.append(blk)

    def blob(self):
        b = np.concatenate(self.cols, axis=1) if self.cols else np.zeros((self.rows, 1), np.float32)
        return b.astype(np.float32 if self.dtype == f32 else bf)


# ---------------------------------------------------------------------------
# the launches
# ---------------------------------------------------------------------------
def _load_consts(nc, sc, cf_blob_shape, cb_blob_shape):
    cst = nc.dram_tensor("consts_f", list(cf_blob_shape), f32,
                         kind="ExternalInput").ap()
    cstb = nc.dram_tensor("consts_b", list(cb_blob_shape), bf16,
                          kind="ExternalInput").ap()
    cw = sc.tile(list(cf_blob_shape), f32)
    nc.sync.dma_start(out=cw[:], in_=cst[:])
    cwb = sc.tile(list(cb_blob_shape), bf16)
    nc.sync.dma_start(out=cwb[:], in_=cstb[:])
    return cw, cwb


def _edge_phase(nc, sb, ps, cfv, cbv, S_round, cap, q2c,
                y_maker, t_em_sl, m_em_sl, q_consts, aggp, tag):
    """Common per-round tail: given feature-major y-rows producer and stats in
    s_em form, compute alpha, relu, aggregate into aggp [P, EMB] psum.
    y_maker yields (ysb_slab_tile, j0, w) feature-major [64, w] bf16 slabs and
    stats s_em [P, 4*cap] (s3c, s4c, s1, s2 order per chunk) -- built by
    caller; here we just consume. This helper is inlined in callers instead
    for clarity."""
    raise NotImplementedError


def build_l1(shapes, WB):
    S = shapes["S_vc"]
    caps = shapes["caps_vc"]
    n_rounds = len(caps)
    n_ct = shapes["n_ct"]
    n_vt = shapes["n_vt"]
    CF, CB = shapes["CF"], shapes["CB"]

    nc = bacc.Bacc("TRN2", target_bir_lowering=False, debug=False,
                   num_devices=NCORES)
    x_fm = nc.dram_tensor("x_fm", [1, S], bf16, kind="ExternalInput").ap()
    t_fm = nc.dram_tensor("t_fm", [1, S], f32, kind="ExternalInput").ap()
    t_em = nc.dram_tensor("t_em", [P, S // P], f32, kind="ExternalInput").ap()
    m_em = nc.dram_tensor("m_em", [P, S // P], f32, kind="ExternalInput").ap()
    xc_fm = nc.dram_tensor("xc_fm", [1, n_ct], bf16, kind="ExternalInput").ap()
    mc_fm = nc.dram_tensor("mc_fm", [1, n_ct], bf16, kind="ExternalInput").ap()
    xv_fm = nc.dram_tensor("xv_fm", [1, n_vt], bf16, kind="ExternalInput").ap()
    mv_fm = nc.dram_tensor("mv_fm", [1, n_vt], bf16, kind="ExternalInput").ap()
    a2_tab = nc.dram_tensor("a2_tab", [EMB + 2, n_ct], f32,
                            kind="ExternalOutput").ap()
    vemb_tab = nc.dram_tensor("vemb_tab", [EMB, n_vt], bf16,
                              kind="ExternalOutput").ap()
    cemb_tab = nc.dram_tensor("cemb_tab", [EMB, n_ct], bf16,
                              kind="Internal").ap()

    with tile.TileContext(nc) as tc, ExitStack() as ctx0:
        sb = ctx0.enter_context(tc.tile_pool(name="sb", bufs=2))
        sc = ctx0.enter_context(tc.tile_pool(name="sc", bufs=1))
        ps = ctx0.enter_context(tc.tile_pool(name="ps", bufs=2, space="PSUM"))
        ps2 = ctx0.enter_context(tc.tile_pool(name="ps2", bufs=1, space="PSUM"))

        cw, cwb = _load_consts(nc, sc, shapes["cf_shape"], shapes["cb_shape"])

        def cf(name):
            c0, w, h = CF[name]
            return cw[0:h, c0:c0 + w]

        def cb(name):
            c0, w, h = CB[name]
            return cwb[0:h, c0:c0 + w]

        # ---- node embedding tables (masked) ----
        for (xd, md, od, n_n, w1, b1, w2t, tg) in (
                (xv_fm, mv_fm, vemb_tab, n_vt, "vW1", "vB1", "vW2T", "ev"),
                (xc_fm, mc_fm, cemb_tab, n_ct, "cW1", "cB1", "cW2T", "ec")):
            for j0, w in _slabs(n_n):
                xr = sb.tile([1, SLAB], bf16, tag=f"{tg}x")
                nc.gpsimd.dma_start(out=xr[:, :w], in_=xd[:, j0:j0 + w])
                mr = sb.tile([1, SLAB], bf16, tag=f"{tg}m")
                nc.gpsimd.dma_start(out=mr[:, :w], in_=md[:, j0:j0 + w])
                xb = ps2.tile([EMB, SLAB], f32, space="PSUM", tag="slabp")
                nc.tensor.matmul(out=xb[:, :w], lhsT=cb("ones1"),
                                 rhs=xr[:, :w], start=True, stop=True)
                e1 = sb.tile([EMB, SLAB], bf16, tag=f"{tg}1")
                nc.scalar.activation(out=e1[:, :w], in_=xb[:, :w],
                                     func=Relu, scale=cf(w1), bias=cf(b1))
                e2p = ps2.tile([EMB, SLAB], f32, space="PSUM", tag="slabp")
                nc.tensor.matmul(out=e2p[:, :w], lhsT=cb(w2t), rhs=e1[:, :w],
                                 start=True, stop=True)
                e2 = sb.tile([EMB, SLAB], bf16, tag=f"{tg}2")
                nc.scalar.activation(out=e2[:, :w], in_=e2p[:, :w], func=Relu)
                mpb = ps2.tile([EMB, SLAB], f32, space="PSUM", tag="slabp")
                nc.tensor.matmul(out=mpb[:, :w], lhsT=cb("ones1"),
                                 rhs=mr[:, :w], start=True, stop=True)
                em = sb.tile([EMB, SLAB], bf16, tag=f"{tg}e")
                nc.vector.tensor_tensor(out=em[:, :w], in0=e2[:, :w],
                                        in1=mpb[:, :w], op=MULT)
                nc.sync.dma_start(out=od[:, j0:j0 + w], in_=em[:, :w])

        # ---- conv v->c per round ----
        soff = 0
        for r in range(n_rounds):
            cap = int(caps[r])
            Sg = cap * P
            co = soff // P
            t_eml = sb.tile([P, cap], f32, tag="teml")
            nc.sync.dma_start(out=t_eml[:], in_=t_em[:, co:co + cap])
            m_eml = sb.tile([P, cap], f32, tag="meml")
            nc.sync.dma_start(out=m_eml[:], in_=m_em[:, co:co + cap])
            x_sl = sb.tile([1, Sg], bf16, tag="xsl")
            nc.gpsimd.dma_start(out=x_sl[:], in_=x_fm[:, soff:soff + Sg])
            t_sl = sb.tile([1, Sg], f32, tag="tsl")
            nc.sync.dma_start(out=t_sl[:], in_=t_fm[:, soff:soff + Sg])
            tb_sl = sb.tile([1, Sg], bf16, tag="tbsl")
            nc.vector.tensor_copy(out=tb_sl[:], in_=t_sl[:])

            s_em = sb.tile([P, 4 * cap], f32, tag="sem")
            zpall = ps.tile([P, Sg // 2], bf16, space="PSUM", tag="zpall", bufs=1)

            for j0, w in _slabs(Sg):
                xb = ps2.tile([EMB, SLAB], f32, space="PSUM", tag="slabp")
                nc.tensor.matmul(out=xb[:, :w], lhsT=cb("ones1"),
                                 rhs=x_sl[:, j0:j0 + w], start=True, stop=True)
                e1 = sb.tile([EMB, SLAB], bf16, tag="ve1")
                nc.scalar.activation(out=e1[:, :w], in_=xb[:, :w],
                                     func=Relu, scale=cf("vW1"), bias=cf("vB1"))
                e2p = ps2.tile([EMB, SLAB], f32, space="PSUM", tag="slabp")
                nc.tensor.matmul(out=e2p[:, :w], lhsT=cb("vW2T"), rhs=e1[:, :w],
                                 start=True, stop=True)
                e2 = sb.tile([EMB, SLAB], bf16, tag="ve2")
                nc.scalar.activation(out=e2[:, :w], in_=e2p[:, :w], func=Relu)
                ap_ = ps2.tile([EMB, SLAB], f32, space="PSUM", tag="slaba")
                nc.tensor.matmul(out=ap_[:, :w], lhsT=cb("vWl"), rhs=e2[:, :w],
                                 start=True, stop=False)
                y0 = sb.tile([EMB, SLAB], bf16, tag="vy0")
                nc.scalar.activation(out=y0[:, :w], in_=ap_[:, :w], func=Copy)
                sq = sb.tile([EMB, SLAB], bf16, tag="vsq")
                nc.vector.tensor_tensor(out=sq[:, :w], in0=y0[:, :w],
                                        in1=y0[:, :w], op=MULT)
                stp = ps.tile([3, SLAB], f32, space="PSUM", tag="smallp")
                nc.tensor.matmul(out=stp[0:3, :w], lhsT=cb("vstat3"),
                                 rhs=y0[:, :w], start=True, stop=True)
                stp2 = ps.tile([1, SLAB], f32, space="PSUM", tag="smallp")
                nc.tensor.matmul(out=stp2[0:1, :w], lhsT=cb("ones64d"),
                                 rhs=sq[:, :w], start=True, stop=True)
                sts = sb.tile([3, SLAB], f32, tag="vsts")
                nc.vector.tensor_copy(out=sts[0:3, :w], in_=stp[0:3, :w])
                sts2 = sb.tile([1, SLAB], f32, tag="vsts2")
                nc.vector.tensor_copy(out=sts2[0:1, :w], in_=stp2[0:1, :w])
                # mu row (s1) -> bf16 for rank-1
                mub = sb.tile([1, SLAB], bf16, tag="vmub")
                nc.vector.tensor_copy(out=mub[:, :w], in_=sts[0:1, :w])
                nc.tensor.matmul(out=ap_[:, :w], lhsT=cb("negones1"),
                                 rhs=mub[:, :w], start=False, stop=False)
                nc.tensor.matmul(out=ap_[:, :w], lhsT=cb("what1_vc"),
                                 rhs=tb_sl[:, j0:j0 + w], start=False, stop=True)
                ysb = sb.tile([EMB, SLAB], bf16, tag="vysb")
                nc.scalar.activation(out=ysb[:, :w], in_=ap_[:, :w],
                                     func=Identity, bias=cf("bhat_vc"))
                for cc in range(w // P):
                    c = (j0 + cc * P) // P
                    zslice = bass.AP(zpall.tensor, zpall.offset + c * EMB * 2,
                                     [(zpall.ap[0][0], P), (1, EMB)]) if False \
                        else zpall[:, c * EMB:(c + 1) * EMB]
                    nc.tensor.transpose(out=zslice,
                                        in_=ysb[:, cc * P:(cc + 1) * P],
                                        identity=cb("identEb"))
                # stats transposes per chunk
                for cc in range(w // P):
                    c = (j0 + cc * P) // P
                    tp = ps.tile([P, 4], f32, space="PSUM", tag="smallp")
                    nc.tensor.transpose(out=tp[:, 0:3],
                                        in_=sts[0:3, cc * P:(cc + 1) * P],
                                        identity=cf("ident4")[0:3, 0:3])
                    nc.tensor.transpose(out=tp[:, 3:4],
                                        in_=sts2[0:1, cc * P:(cc + 1) * P],
                                        identity=cf("ident4")[0:1, 0:1])
                    nc.vector.tensor_copy(out=s_em[:, c * 4:(c + 1) * 4],
                                          in_=tp[:])
            # alpha pipe [P, cap]
            sv = s_em[:].rearrange("p (c k) -> p c k", c=cap, k=4)
            al = _alpha(nc, sb, cf, sv, t_eml, m_eml, cap,
                        WB["q2_vc"], WB["c_bb_vc"], WB["c_bw_vc"], "va")
            # relu + alpha + aggregate
            zeall = sb.tile([P, Sg // 2], bf16, tag="zeall")
            nc.scalar.activation(out=zeall[:], in_=zpall[:], func=Relu)
            zs = sb.tile([P, Sg // 2], bf16, tag="zsall")
            alv = al[:, :, None].broadcast_to([P, cap, EMB])
            zev = zeall[:].rearrange("p (c f) -> p c f", c=cap, f=EMB)
            zsv = zs[:].rearrange("p (c f) -> p c f", c=cap, f=EMB)
            nc.vector.tensor_tensor(out=zsv, in0=zev, in1=alv, op=MULT)
            aggp = ps.tile([P, EMB], f32, space="PSUM", tag="aggp", bufs=1)
            for c in range(cap):
                nc.tensor.matmul(out=aggp[:], lhsT=cb("ident128"),
                                 rhs=zs[:, c * EMB:(c + 1) * EMB],
                                 start=(c == 0), stop=(c == cap - 1))
            # ---- node MLP + A2 table for this tile ----
            ag_sb = sb.tile([P, EMB], bf16, tag="nagsb")
            nc.vector.tensor_copy(out=ag_sb[:], in_=aggp[:])
            agT_p = ps.tile([EMB, P], bf16, space="PSUM", tag="nodep", bufs=1)
            nc.tensor.transpose(out=agT_p[:], in_=ag_sb[:],
                                identity=cb("ident128"))
            agT = sb.tile([EMB, P], bf16, tag="nagt")
            nc.vector.tensor_copy(out=agT[:], in_=agT_p[:])
            right = sb.tile([EMB, P], bf16, tag="nright")
            nc.sync.dma_start(out=right[:], in_=cemb_tab[:, r * P:(r + 1) * P])
            up = ps.tile([EMB, P], f32, space="PSUM", tag="nodep", bufs=1)
            nc.tensor.matmul(out=up[:], lhsT=cb("M1_vc"), rhs=agT[:],
                             start=True, stop=False)
            nc.tensor.matmul(out=up[:], lhsT=cb("M2_vc"), rhs=right[:],
                             start=False, stop=True)
            u = sb.tile([EMB, P], bf16, tag="nu")
            nc.scalar.activation(out=u[:], in_=up[:], func=Relu)
            cnp = ps.tile([EMB, P], f32, space="PSUM", tag="nodep", bufs=1)
            nc.tensor.matmul(out=cnp[:], lhsT=cb("Wo2_vc"), rhs=u[:],
                             start=True, stop=True)
            cn = sb.tile([EMB, P], bf16, tag="ncn")
            nc.scalar.activation(out=cn[:], in_=cnp[:], func=Copy)
            a2p = ps.tile([EMB + 1, P], f32, space="PSUM", tag="nodep", bufs=1)
            nc.tensor.matmul(out=a2p[:], lhsT=cb("cvWlmu"), rhs=cn[:],
                             start=True, stop=False)
            mu2r = sb.tile([1, P], bf16, tag="nmu2")
            nc.vector.tensor_copy(out=mu2r[:], in_=a2p[EMB:EMB + 1, :])
            nc.tensor.matmul(out=a2p[0:EMB, :], lhsT=cb("negones1"),
                             rhs=mu2r[:], start=False, stop=True,
                             skip_group_check=True)
            u2 = sb.tile([EMB, P], f32, tag="nu2")
            nc.scalar.activation(out=u2[:], in_=a2p[0:EMB, :], func=Identity,
                                 bias=cf("bhat_cv"))
            u2b = sb.tile([EMB, P], bf16, tag="nu2b")
            nc.vector.tensor_copy(out=u2b[:], in_=u2[:])
            u2sq = sb.tile([EMB, P], bf16, tag="nu2sq")
            nc.vector.tensor_tensor(out=u2sq[:], in0=u2b[:], in1=u2b[:], op=MULT)
            qp = ps.tile([1, P], f32, space="PSUM", tag="smallp")
            nc.tensor.matmul(out=qp[0:1, :], lhsT=cb("ones64d"), rhs=u2sq[:],
                             start=True, stop=True)
            qs0 = sb.tile([1, P], f32, tag="nqs0")
            nc.vector.tensor_copy(out=qs0[:], in_=qp[0:1, :])
            qp2 = ps.tile([1, P], f32, space="PSUM", tag="smallp")
            nc.tensor.matmul(out=qp2[0:1, :], lhsT=cb("what2d"), rhs=u2b[:],
                             start=True, stop=True)
            qs1 = sb.tile([1, P], f32, tag="nqs1")
            nc.vector.tensor_copy(out=qs1[:], in_=qp2[0:1, :])
            nc.sync.dma_start(out=a2_tab[0:EMB, r * P:(r + 1) * P], in_=u2[:])
            nc.sync.dma_start(out=a2_tab[EMB:EMB + 1, r * P:(r + 1) * P],
                              in_=qs0[:])
            nc.sync.dma_start(out=a2_tab[EMB + 1:EMB + 2, r * P:(r + 1) * P],
                              in_=qs1[:])
            soff += Sg

    nc.compile()
    return nc


def _alpha(nc, sb, cf, sv, t_eml, m_eml, cap, q2, c_bb, c_bw, tg):
    """var = (s2 - s1^2 + s3 + c_bb) + t*(s4 + 2*c_bw) + t^2*q2; al = m/sqrt(var+eps)
    sv: [P, cap, 4] strided stats view (s3, s4, s1, s2)."""
    s3c = sb.tile([P, cap], f32, tag=f"{tg}3")
    nc.vector.tensor_scalar(out=s3c[:], in0=sv[:, :, 1], scalar1=float(c_bb),
                            scalar2=None, op0=ADD)
    s4c = sb.tile([P, cap], f32, tag=f"{tg}4")
    nc.vector.tensor_scalar(out=s4c[:], in0=sv[:, :, 2], scalar1=float(2 * c_bw),
                            scalar2=None, op0=ADD)
    v1 = sb.tile([P, cap], f32, tag=f"{tg}v1")
    nc.vector.scalar_tensor_tensor(out=v1[:], in0=t_eml[:], scalar=float(q2),
                                   in1=s4c[:], op0=MULT, op1=ADD)
    v2 = sb.tile([P, cap], f32, tag=f"{tg}v2")
    nc.vector.tensor_tensor(out=v2[:], in0=v1[:], in1=t_eml[:], op=MULT)
    v3 = sb.tile([P, cap], f32, tag=f"{tg}v3")
    nc.vector.tensor_tensor(out=v3[:], in0=sv[:, :, 0], in1=sv[:, :, 0], op=MULT)
    v4 = sb.tile([P, cap], f32, tag=f"{tg}v4")
    nc.vector.tensor_tensor(out=v4[:], in0=sv[:, :, 3], in1=v3[:], op=SUB)
    v5 = sb.tile([P, cap], f32, tag=f"{tg}v5")
    nc.vector.tensor_tensor(out=v5[:], in0=v4[:], in1=s3c[:], op=ADD)
    var = sb.tile([P, cap], f32, tag=f"{tg}vr")
    nc.vector.tensor_tensor(out=var[:], in0=v5[:], in1=v2[:], op=ADD)
    sd = sb.tile([P, cap], f32, tag=f"{tg}sd")
    nc.scalar.activation(out=sd[:], in_=var[:], func=Sqrt, bias=cf("eps"),
                         scale=1.0)
    rs = sb.tile([P, cap], f32, tag=f"{tg}rs")
    nc.vector.reciprocal(out=rs[:], in_=sd[:])
    al = sb.tile([P, cap], f32, tag=f"{tg}al")
    nc.vector.tensor_tensor(out=al[:], in0=rs[:], in1=m_eml[:], op=MULT)
    alb = sb.tile([P, cap], bf16, tag=f"{tg}ab")
    nc.vector.tensor_copy(out=alb[:], in_=al[:])
    return alb


def build_l2(shapes, WB):
    S = shapes["S_cv"]
    caps = shapes["caps_cv"]
    n_rounds = len(caps)
    n_vt = shapes["n_vt"]
    CF, CB = shapes["CF"], shapes["CB"]

    nc = bacc.Bacc("TRN2", target_bir_lowering=False, debug=False,
                   num_devices=NCORES)
    u2_fm = nc.dram_tensor("u2_fm", [EMB, S], bf16, kind="ExternalInput").ap()
    t_fm = nc.dram_tensor("t2_fm", [1, S], bf16, kind="ExternalInput").ap()
    t_em = nc.dram_tensor("t2_em", [P, S // P], f32, kind="ExternalInput").ap()
    m_em = nc.dram_tensor("m2_em", [P, S // P], f32, kind="ExternalInput").ap()
    q0_em = nc.dram_tensor("q0_em", [P, S // P], f32, kind="ExternalInput").ap()
    q1_em = nc.dram_tensor("q1_em", [P, S // P], f32, kind="ExternalInput").ap()
    vemb_tab = nc.dram_tensor("vemb_tab", [EMB, n_vt], bf16,
                              kind="ExternalInput").ap()
    head_out = nc.dram_tensor("head_out", [1, 1], f32, kind="ExternalOutput").ap()

    with tile.TileContext(nc) as tc, ExitStack() as ctx0:
        sb = ctx0.enter_context(tc.tile_pool(name="sb", bufs=2))
        sc = ctx0.enter_context(tc.tile_pool(name="sc", bufs=1))
        ps = ctx0.enter_context(tc.tile_pool(name="ps", bufs=1, space="PSUM"))
        ps2 = ctx0.enter_context(tc.tile_pool(name="ps2", bufs=1, space="PSUM"))

        cw, cwb = _load_consts(nc, sc, shapes["cf_shape"], shapes["cb_shape"])

        def cf(name):
            c0, w, h = CF[name]
            return cw[0:h, c0:c0 + w]

        def cb(name):
            c0, w, h = CB[name]
            return cwb[0:h, c0:c0 + w]

        headp = ps.tile([1, P], f32, space="PSUM", tag="headp", bufs=1)
        soff = 0
        for r in range(n_rounds):
            cap = int(caps[r])
            Sg = cap * P
            co = soff // P
            t_eml = sb.tile([P, cap], f32, tag="teml")
            nc.sync.dma_start(out=t_eml[:], in_=t_em[:, co:co + cap])
            m_eml = sb.tile([P, cap], f32, tag="meml")
            nc.sync.dma_start(out=m_eml[:], in_=m_em[:, co:co + cap])
            q0l = sb.tile([P, cap], f32, tag="q0l")
            nc.sync.dma_start(out=q0l[:], in_=q0_em[:, co:co + cap])
            q1l = sb.tile([P, cap], f32, tag="q1l")
            nc.sync.dma_start(out=q1l[:], in_=q1_em[:, co:co + cap])
            t_sl = sb.tile([1, Sg], bf16, tag="tsl")
            nc.gpsimd.dma_start(out=t_sl[:], in_=t_fm[:, soff:soff + Sg])
            u2l = sb.tile([EMB, Sg], bf16, tag="u2l")
            nc.sync.dma_start(out=u2l[:], in_=u2_fm[:, soff:soff + Sg])

            # y2 = u2 + t*what2 (rank-1 psum + add, per slab)
            y2 = sb.tile([EMB, Sg], bf16, tag="y2")
            for j0, w in _slabs(Sg):
                twp = ps2.tile([EMB, SLAB], f32, space="PSUM", tag="slabp")
                nc.tensor.matmul(out=twp[:, :w], lhsT=cb("what1_cv"),
                                 rhs=t_sl[:, j0:j0 + w], start=True, stop=True)
                nc.vector.tensor_tensor(out=y2[:, j0:j0 + w],
                                        in0=u2l[:, j0:j0 + w],
                                        in1=twp[:, :w], op=ADD)

            zpall = ps.tile([P, Sg // 2], bf16, space="PSUM", tag="zpall", bufs=1)
            for c in range(cap):
                nc.tensor.transpose(out=zpall[:, c * EMB:(c + 1) * EMB],
                                    in_=y2[:, c * P:(c + 1) * P],
                                    identity=cb("identEb"))
            # alpha2: var = q0 + t*q1 + t^2 q2
            v1 = sb.tile([P, cap], f32, tag="cv1")
            nc.vector.scalar_tensor_tensor(out=v1[:], in0=t_eml[:],
                                           scalar=float(WB["q2_cv"]),
                                           in1=q1l[:], op0=MULT, op1=ADD)
            v2 = sb.tile([P, cap], f32, tag="cv2")
            nc.vector.tensor_tensor(out=v2[:], in0=v1[:], in1=t_eml[:], op=MULT)
            var = sb.tile([P, cap], f32, tag="cvr")
            nc.vector.tensor_tensor(out=var[:], in0=v2[:], in1=q0l[:], op=ADD)
            sd = sb.tile([P, cap], f32, tag="csd")
            nc.scalar.activation(out=sd[:], in_=var[:], func=Sqrt,
                                 bias=cf("eps"), scale=1.0)
            rs = sb.tile([P, cap], f32, tag="crs")
            nc.vector.reciprocal(out=rs[:], in_=sd[:])
            al = sb.tile([P, cap], f32, tag="cal")
            nc.vector.tensor_tensor(out=al[:], in0=rs[:], in1=m_eml[:], op=MULT)
            alb = sb.tile([P, cap], bf16, tag="cab")
            nc.vector.tensor_copy(out=alb[:], in_=al[:])

            zeall = sb.tile([P, Sg // 2], bf16, tag="zeall")
            nc.scalar.activation(out=zeall[:], in_=zpall[:], func=Relu)
            zs = sb.tile([P, Sg // 2], bf16, tag="zsall")
            alv = alb[:, :, None].broadcast_to([P, cap, EMB])
            zev = zeall[:].rearrange("p (c f) -> p c f", c=cap, f=EMB)
            zsv = zs[:].rearrange("p (c f) -> p c f", c=cap, f=EMB)
            nc.vector.tensor_tensor(out=zsv, in0=zev, in1=alv, op=MULT)
            aggp = ps.tile([P, EMB], f32, space="PSUM", tag="aggp", bufs=1)
            for c in range(cap):
                nc.tensor.matmul(out=aggp[:], lhsT=cb("ident128"),
                                 rhs=zs[:, c * EMB:(c + 1) * EMB],
                                 start=(c == 0), stop=(c == cap - 1))
            # node MLP + head
            ag_sb = sb.tile([P, EMB], bf16, tag="nagsb")
            nc.vector.tensor_copy(out=ag_sb[:], in_=aggp[:])
            agT_p = ps.tile([EMB, P], bf16, space="PSUM", tag="nodep", bufs=1)
            nc.tensor.transpose(out=agT_p[:], in_=ag_sb[:],
                                identity=cb("ident128"))
            agT = sb.tile([EMB, P], bf16, tag="nagt")
            nc.vector.tensor_copy(out=agT[:], in_=agT_p[:])
            right = sb.tile([EMB, P], bf16, tag="nright")
            nc.sync.dma_start(out=right[:], in_=vemb_tab[:, r * P:(r + 1) * P])
            up = ps.tile([EMB, P], f32, space="PSUM", tag="nodep", bufs=1)
            nc.tensor.matmul(out=up[:], lhsT=cb("M1_cv"), rhs=agT[:],
                             start=True, stop=False)
            nc.tensor.matmul(out=up[:], lhsT=cb("M2_cv"), rhs=right[:],
                             start=False, stop=True)
            u = sb.tile([EMB, P], bf16, tag="nu")
            nc.scalar.activation(out=u[:], in_=up[:], func=Relu)
            vnp = ps.tile([EMB, P], f32, space="PSUM", tag="nodep", bufs=1)
            nc.tensor.matmul(out=vnp[:], lhsT=cb("Wo2_cv"), rhs=u[:],
                             start=True, stop=True)
            vn = sb.tile([EMB, P], bf16, tag="nvn")
            nc.scalar.activation(out=vn[:], in_=vnp[:], func=Copy)
            pp = ps.tile([EMB, P], f32, space="PSUM", tag="nodep", bufs=1)
            nc.tensor.matmul(out=pp[:], lhsT=cb("Wp1"), rhs=vn[:],
                             start=True, stop=True)
            pr = sb.tile([EMB, P], bf16, tag="npr")
            nc.scalar.activation(out=pr[:], in_=pp[:], func=Relu)
            nc.tensor.matmul(out=headp[:], lhsT=cb("Wp2col"), rhs=pr[:],
                             start=(r == 0), stop=(r == n_rounds - 1))
            soff += Sg

        hsb = sb.tile([1, P], f32, tag="hsb")
        nc.vector.tensor_copy(out=hsb[:], in_=headp[:])
        hred = sb.tile([1, 1], f32, tag="hred")
        nc.vector.tensor_reduce(out=hred[:], in_=hsb[:],
                                axis=mybir.AxisListType.X, op=ADD)
        nc.sync.dma_start(out=head_out[:], in_=hred[:])

    nc.compile()
    return nc


# ---------------------------------------------------------------------------
# host orchestration
# ---------------------------------------------------------------------------
def kernel(constraint_features, edge_indices, edge_features, variable_features,
           cons_emb, var_emb, conv_vc, conv_cv, out_mlp):
    from concourse.bass_utils import run_bass_kernel_spmd

    NC_ = constraint_features.shape[0]
    NV_ = variable_features.shape[0]
    ci = np.asarray(edge_indices[0], np.int64)
    vi = np.asarray(edge_indices[1], np.int64)
    xc = np.asarray(constraint_features, np.float32).reshape(-1)
    xv = np.asarray(variable_features, np.float32).reshape(-1)
    ef = np.asarray(edge_features, np.float32).reshape(-1)

    cW1, cW2 = [np.asarray(a, np.float32) for a in cons_emb]
    vW1, vW2 = [np.asarray(a, np.float32) for a in var_emb]
    (vc_Wl, vc_bl, vc_We, vc_be, vc_g, vc_b, vc_Wf, vc_Wo1, vc_Wo2) = \
        [np.asarray(a, np.float32) for a in conv_vc]
    (cv_Wl, cv_bl, cv_We, cv_be, cv_g, cv_b, cv_Wf, cv_Wo1, cv_Wo2) = \
        [np.asarray(a, np.float32) for a in conv_cv]
    Wp1, Wp2 = [np.asarray(a, np.float32) for a in out_mlp]
    assert np.all(vc_g == 1) and np.all(vc_b == 0)
    assert np.all(cv_g == 1) and np.all(cv_b == 0)

    w_vc = vc_We.reshape(-1)
    what_vc = w_vc - w_vc.mean()
    b_vc = vc_bl + vc_be
    bhat_vc = b_vc - b_vc.mean()
    w_cv = cv_We.reshape(-1)
    what_cv = w_cv - w_cv.mean()
    b_cv = cv_bl + cv_be
    bhat_cv = b_cv - b_cv.mean()
    WB = {
        "q2_vc": float((what_vc ** 2).mean()),
        "c_bb_vc": float((bhat_vc ** 2).mean()),
        "c_bw_vc": float((bhat_vc * what_vc).mean()),
        "q2_cv": float((what_cv ** 2).mean()),
    }

    plan_vc = ConvPlan(ci, NC_)
    plan_cv = ConvPlan(vi, NV_)

    # ---- constants ----
    cfp = ConstPack(P, f32)
    cfp.add("vW1", vW1.reshape(-1))          # [64]
    cfp.add("vB1", np.zeros(EMB))            # no bias in embed lin1
    cfp.add("cW1", cW1.reshape(-1))
    cfp.add("cB1", np.zeros(EMB))
    cfp.add("bhat_vc", bhat_vc)
    cfp.add("bhat_cv", bhat_cv)
    cfp.add("eps", np.full(P, LN_EPS))
    cfp.add("what2col", what_cv)
    cfp.add("ident4", np.eye(4))
    cf_blob = cfp.blob()

    cbp = ConstPack(P, bf16)
    cbp.add("vW2T", vW2)                     # lhsT [in,out] as stored
    cbp.add("cW2T", cW2)
    cbp.add("vWl", vc_Wl)
    cbp.add("vstat3", np.stack([np.ones(EMB) / EMB, 2 * bhat_vc / EMB,
                                2 * what_vc / EMB], axis=1))
    cbp.add("ones64d", np.ones(EMB)[:, None] / EMB)
    cbp.add("negones1", -np.ones((1, EMB)))
    cbp.add("ones1", np.ones((1, EMB)))
    cbp.add("what1_vc", what_vc[None, :])
    cbp.add("what1_cv", what_cv[None, :])
    cbp.add("identEb", np.eye(EMB))
    cbp.add("ident128", np.eye(P))
    cbp.add("M1_vc", vc_Wf @ vc_Wo1[:EMB])
    cbp.add("M2_vc", vc_Wo1[EMB:])
    cbp.add("Wo2_vc", vc_Wo2)
    cbp.add("cvWlmu", np.concatenate([cv_Wl, cv_Wl.mean(axis=1, keepdims=True)],
                                     axis=1))
    cbp.add("what2d", (2 * what_cv / EMB)[:, None])
    cbp.add("M1_cv", cv_Wf @ cv_Wo1[:EMB])
    cbp.add("M2_cv", cv_Wo1[EMB:])
    cbp.add("Wo2_cv", cv_Wo2)
    cbp.add("Wp1", Wp1)
    cbp.add("Wp2col", Wp2.reshape(EMB, 1))
    cb_blob = cbp.blob()

    # ---- streams L1 (conv v->c) ----
    x_slot = plan_vc.scatter(xv[vi])
    t_slot = plan_vc.scatter(ef)
    mask_ = np.zeros(len(ef))
    m_slot = plan_vc.scatter(np.ones(len(ef)))
    n_ct = plan_vc.n_rounds * P
    n_vt = plan_cv.n_rounds * P

    xc_perm = np.zeros((NCORES, n_ct), np.float32)
    mc_perm = np.zeros((NCORES, n_ct), np.float32)
    xv_perm = np.zeros((NCORES, n_vt), np.float32)
    mv_perm = np.zeros((NCORES, n_vt), np.float32)
    crows, vrows = [], []
    for c in range(NCORES):
        cr = plan_vc.target_rows(c)
        vr = plan_cv.target_rows(c)
        crows.append(cr)
        vrows.append(vr)
        vc_valid = cr >= 0
        xc_perm[c, vc_valid] = xc[cr[vc_valid]]
        mc_perm[c, vc_valid] = 1.0
        vv = vr >= 0
        xv_perm[c, vv] = xv[vr[vv]]
        mv_perm[c, vv] = 1.0

    shapes = {
        "S_vc": plan_vc.total_slots, "caps_vc": plan_vc.caps,
        "S_cv": plan_cv.total_slots, "caps_cv": plan_cv.caps,
        "n_ct": n_ct, "n_vt": n_vt,
        "CF": cfp.off, "CB": cbp.off,
        "cf_shape": cf_blob.shape, "cb_shape": cb_blob.shape,
    }

    nc1 = build_l1(shapes, WB)
    in_maps1 = []
    for c in range(NCORES):
        in_maps1.append({
            "consts_f": cf_blob, "consts_b": cb_blob,
            "x_fm": x_slot[c][None, :].astype(bf), "t_fm": t_slot[c][None, :],
            "t_em": t_slot[c].reshape(-1, P).T.copy(),
            "m_em": m_slot[c].reshape(-1, P).T.copy(),
            "xc_fm": xc_perm[c][None, :].astype(bf), "mc_fm": mc_perm[c][None, :].astype(bf),
            "xv_fm": xv_perm[c][None, :].astype(bf), "mv_fm": mv_perm[c][None, :].astype(bf),
        })
    res1 = run_bass_kernel_spmd(nc1, in_maps1, list(range(NCORES)))

    # assemble tables: a2 rows per constraint id; vemb per core
    a2_full = np.zeros((EMB + 2, NC_), np.float32)
    for c in range(NCORES):
        tab = np.asarray(res1.results[c]["a2_tab"], np.float32)
        cr = crows[c]
        valid = cr >= 0
        a2_full[:, cr[valid]] = tab[:, valid]
    vembs = [np.asarray(res1.results[c]["vemb_tab"]) for c in range(NCORES)]

    # ---- streams L2 (conv c->v): expand a2 table per slot ----
    src_slot = plan_cv.scatter(ci.astype(np.float64)).astype(np.int64)
    # note: scatter fills float; redo exactly:
    src_slot = np.zeros((NCORES, plan_cv.total_slots), np.int64)
    src_slot[plan_cv.edge_core, plan_cv.edge_pos] = ci
    t2_slot = plan_cv.scatter(ef)
    m2_slot = plan_cv.scatter(np.ones(len(ef)))

    nc2 = build_l2(shapes, WB)
    in_maps2 = []
    for c in range(NCORES):
        rows = a2_full[:, src_slot[c]]                  # [66, S_cv]
        u2 = rows[0:EMB].astype(bf)
        q0 = rows[EMB].astype(np.float32)
        q1 = rows[EMB + 1].astype(np.float32)
        in_maps2.append({
            "consts_f": cf_blob, "consts_b": cb_blob,
            "u2_fm": u2,
            "t2_fm": t2_slot[c][None, :].astype(bf),
            "t2_em": t2_slot[c].reshape(-1, P).T.copy(),
            "m2_em": m2_slot[c].reshape(-1, P).T.copy(),
            "q0_em": q0.reshape(-1, P).T.copy(),
            "q1_em": q1.reshape(-1, P).T.copy(),
            "vemb_tab": vembs[c],
        })
    res2 = run_bass_kernel_spmd(nc2, in_maps2, list(range(NCORES)))

    total = sum(float(np.asarray(res2.results[c]["head_out"])[0, 0])
                for c in range(NCORES))
    out = 1.0 / (1.0 + np.exp(-(total / NV_)))
    return np.array([out], np.float32)


# revision 7
# speedup vs baseline: 1.0974x; 1.0974x over previous
"""GNNPolicy bipartite-GNN kernel for 8 TRN2 NeuronCores (Bass/Tile).

See module notes at bottom. kernel(**inputs) -> np.ndarray [1].
Design summary:
 - Aggregation targets degree-sorted into 128-target tiles; tiles grouped in
   rounds of 8 (one per core, shared instruction stream, shared per-round
   slot cap).
 - conv v->c runs entirely from host-prepared per-edge-slot scalar streams
   (variable features are scalars); the embedding MLP runs per slot on device
   (feature-major, ACT-fused).
 - LN decomposes into per-slot scalars (mean/var quadratic in edge feature);
   alpha>0 commutes with relu and is applied as a per-partition scale during
   PSUM-accumulated aggregation.
 - Between launches the host re-shards device-computed per-constraint tables
   into conv c->v per-slot streams (index-space row movement only).
"""
import sys
from contextlib import ExitStack

import numpy as np
import ml_dtypes

if "/opt/trn_rl_repo" not in sys.path:
    sys.path.insert(0, "/opt/trn_rl_repo")

import concourse.bass as bass  # noqa: E402
import concourse.tile as tile  # noqa: E402
from concourse import bacc, mybir  # noqa: E402

f32 = mybir.dt.float32
bf16 = mybir.dt.bfloat16
bf = ml_dtypes.bfloat16

P = 128
NCORES = 8
EMB = 64
LN_EPS = 1e-5
SLAB = 512

Relu = mybir.ActivationFunctionType.Relu
Identity = mybir.ActivationFunctionType.Identity
Sqrt = mybir.ActivationFunctionType.Sqrt
Copy = mybir.ActivationFunctionType.Copy
ADD = mybir.AluOpType.add
SUB = mybir.AluOpType.subtract
MULT = mybir.AluOpType.mult


def ceil_div(a, b):
    return (a + b - 1) // b


def _slabs(n, width=SLAB):
    return [(i, min(width, n - i)) for i in range(0, n, width)]


# ---------------------------------------------------------------------------
# host planning
# ---------------------------------------------------------------------------
class ConvPlan:
    def __init__(self, tgt_idx, n_tgt):
        E = len(tgt_idx)
        deg = np.bincount(tgt_idx, minlength=n_tgt)
        order = np.argsort(-deg, kind="stable")
        n_rounds = ceil_div(n_tgt, NCORES * P)
        n_pad = n_rounds * NCORES * P
        self.n_tgt, self.n_rounds = n_tgt, n_rounds
        self.perm = np.concatenate([order, np.full(n_pad - n_tgt, -1, np.int64)])
        inv = np.empty(n_tgt, np.int64)
        inv[order] = np.arange(n_tgt)
        deg_sorted = np.concatenate([deg[order], np.zeros(n_pad - n_tgt, np.int64)])
        caps = deg_sorted.reshape(n_rounds, NCORES * P).max(axis=1)
        self.caps = np.maximum(caps, 1).astype(np.int64)
        self.total_slots = int(self.caps.sum()) * P
        self.round_off = np.concatenate([[0], np.cumsum(self.caps)[:-1]]) * P

        pos = inv[tgt_idx]
        rnd = pos // (NCORES * P)
        core = (pos // P) % NCORES
        part = pos % P
        sidx = np.argsort(pos, kind="stable")
        psort = pos[sidx]
        starts = np.r_[0, np.where(np.diff(psort) != 0)[0] + 1]
        ranks = np.arange(E) - np.repeat(starts, np.diff(np.r_[starts, E]))
        slot = np.empty(E, np.int64)
        slot[sidx] = ranks
        self.edge_core = core
        self.edge_pos = self.round_off[rnd] + slot * P + part

    def scatter(self, values, dtype=np.float32, fill=0.0):
        out = np.full((NCORES, self.total_slots), fill, dtype)
        out[self.edge_core, self.edge_pos] = values.astype(dtype)
        return out

    def target_rows(self, core):
        ids = np.empty(self.n_rounds * P, np.int64)
        for r in range(self.n_rounds):
            base = r * NCORES * P + core * P
            ids[r * P:(r + 1) * P] = self.perm[base:base + P]
        return ids


# ---------------------------------------------------------------------------
# constants packing
# ---------------------------------------------------------------------------
class ConstPack:
    def __init__(self, rows, dtype):
        self.rows, self.dtype = rows, dtype
        self.cols = []
        self.off = {}

    def add(self, name, arr):
        arr = np.asarray(arr, np.float32)
        if arr.ndim == 1:
            arr = arr[:, None]
        h, w = arr.shape
        blk = np.zeros((self.rows, w), np.float32)
        blk[:h] = arr
        self.off[name] = (sum(c.shape[1] for c in self.cols), w, h)
        self.cols.append(blk)

    def blob(self):
        b = np.concatenate(self.cols, axis=1) if self.cols else np.zeros((self.rows, 1), np.float32)
        return b.astype(np.float32 if self.dtype == f32 else bf)


# ---------------------------------------------------------------------------
# the launches
# ---------------------------------------------------------------------------
def _load_consts(nc, sc, cf_blob_shape, cb_blob_shape):
    cst = nc.dram_tensor("consts_f", list(cf_blob_shape), f32,
                         kind="ExternalInput").ap()
    cstb = nc.dram_tensor("consts_b", list(cb_blob_shape), bf16,
                          kind="ExternalInput").ap()
    cw = sc.tile(list(cf_blob_shape), f32)
    nc.sync.dma_start(out=cw[:], in_=cst[:])
    cwb = sc.tile(list(cb_blob_shape), bf16)
    nc.sync.dma_start(out=cwb[:], in_=cstb[:])
    return cw, cwb


def _edge_phase(nc, sb, ps, cfv, cbv, S_round, cap, q2c,
                y_maker, t_em_sl, m_em_sl, q_consts, aggp, tag):
    """Common per-round tail: given feature-major y-rows producer and stats in
    s_em form, compute alpha, relu, aggregate into aggp [P, EMB] psum.
    y_maker yields (ysb_slab_tile, j0, w) feature-major [64, w] bf16 slabs and
    stats s_em [P, 4*cap] (s3c, s4c, s1, s2 order per chunk) -- built by
    caller; here we just consume. This helper is inlined in callers instead
    for clarity."""
    raise NotImplementedError


def build_l1(shapes, WB):
    S = shapes["S_vc"]
    caps = shapes["caps_vc"]
    n_rounds = len(caps)
    n_ct = shapes["n_ct"]
    n_vt = shapes["n_vt"]
    CF, CB = shapes["CF"], shapes["CB"]

    nc = bacc.Bacc("TRN2", target_bir_lowering=False, debug=False,
                   num_devices=NCORES)
    x_fm = nc.dram_tensor("x_fm", [1, S], bf16, kind="ExternalInput").ap()
    t_fm = nc.dram_tensor("t_fm", [1, S], f32, kind="ExternalInput").ap()
    t_em = nc.dram_tensor("t_em", [P, S // P], f32, kind="ExternalInput").ap()
    m_em = nc.dram_tensor("m_em", [P, S // P], f32, kind="ExternalInput").ap()
    xc_fm = nc.dram_tensor("xc_fm", [1, n_ct], bf16, kind="ExternalInput").ap()
    mc_fm = nc.dram_tensor("mc_fm", [1, n_ct], bf16, kind="ExternalInput").ap()
    xv_fm = nc.dram_tensor("xv_fm", [1, n_vt], bf16, kind="ExternalInput").ap()
    mv_fm = nc.dram_tensor("mv_fm", [1, n_vt], bf16, kind="ExternalInput").ap()
    a2_tab = nc.dram_tensor("a2_tab", [EMB + 2, n_ct], f32,
                            kind="ExternalOutput").ap()
    vemb_tab = nc.dram_tensor("vemb_tab", [EMB, n_vt], bf16,
                              kind="ExternalOutput").ap()
    cemb_tab = nc.dram_tensor("cemb_tab", [EMB, n_ct], bf16,
                              kind="Internal").ap()

    with tile.TileContext(nc) as tc, ExitStack() as ctx0:
        sb = ctx0.enter_context(tc.tile_pool(name="sb", bufs=3))
        sc = ctx0.enter_context(tc.tile_pool(name="sc", bufs=1))
        ps = ctx0.enter_context(tc.tile_pool(name="ps", bufs=2, space="PSUM"))
        ps2 = ctx0.enter_context(tc.tile_pool(name="ps2", bufs=1, space="PSUM"))

        cw, cwb = _load_consts(nc, sc, shapes["cf_shape"], shapes["cb_shape"])

        def cf(name):
            c0, w, h = CF[name]
            return cw[0:h, c0:c0 + w]

        def cb(name):
            c0, w, h = CB[name]
            return cwb[0:h, c0:c0 + w]

        # ---- node embedding tables (masked) ----
        for (xd, md, od, n_n, w1, b1, w2t, tg) in (
                (xv_fm, mv_fm, vemb_tab, n_vt, "vW1", "vB1", "vW2T", "ev"),
                (xc_fm, mc_fm, cemb_tab, n_ct, "cW1", "cB1", "cW2T", "ec")):
            for j0, w in _slabs(n_n):
                xr = sb.tile([1, SLAB], bf16, tag=f"{tg}x")
                nc.gpsimd.dma_start(out=xr[:, :w], in_=xd[:, j0:j0 + w])
                mr = sb.tile([1, SLAB], bf16, tag=f"{tg}m")
                nc.gpsimd.dma_start(out=mr[:, :w], in_=md[:, j0:j0 + w])
                xb = ps2.tile([EMB, SLAB], f32, space="PSUM", tag="slabp")
                nc.tensor.matmul(out=xb[:, :w], lhsT=cb("ones1"),
                                 rhs=xr[:, :w], start=True, stop=True)
                e1 = sb.tile([EMB, SLAB], bf16, tag=f"{tg}1")
                nc.scalar.activation(out=e1[:, :w], in_=xb[:, :w],
                                     func=Relu, scale=cf(w1), bias=cf(b1))
                e2p = ps2.tile([EMB, SLAB], f32, space="PSUM", tag="slabp")
                nc.tensor.matmul(out=e2p[:, :w], lhsT=cb(w2t), rhs=e1[:, :w],
                                 start=True, stop=True)
                e2 = sb.tile([EMB, SLAB], bf16, tag=f"{tg}2")
                nc.scalar.activation(out=e2[:, :w], in_=e2p[:, :w], func=Relu)
                mpb = ps2.tile([EMB, SLAB], f32, space="PSUM", tag="slabp")
                nc.tensor.matmul(out=mpb[:, :w], lhsT=cb("ones1"),
                                 rhs=mr[:, :w], start=True, stop=True)
                em = sb.tile([EMB, SLAB], bf16, tag=f"{tg}e")
                nc.vector.tensor_tensor(out=em[:, :w], in0=e2[:, :w],
                                        in1=mpb[:, :w], op=MULT)
                nc.sync.dma_start(out=od[:, j0:j0 + w], in_=em[:, :w])

        # ---- conv v->c per round ----
        soff = 0
        for r in range(n_rounds):
            cap = int(caps[r])
            Sg = cap * P
            co = soff // P
            t_eml = sb.tile([P, cap], f32, tag="teml")
            nc.sync.dma_start(out=t_eml[:], in_=t_em[:, co:co + cap])
            m_eml = sb.tile([P, cap], f32, tag="meml")
            nc.sync.dma_start(out=m_eml[:], in_=m_em[:, co:co + cap])
            x_sl = sb.tile([1, Sg], bf16, tag="xsl")
            nc.gpsimd.dma_start(out=x_sl[:], in_=x_fm[:, soff:soff + Sg])
            t_sl = sb.tile([1, Sg], f32, tag="tsl")
            nc.sync.dma_start(out=t_sl[:], in_=t_fm[:, soff:soff + Sg])
            tb_sl = sb.tile([1, Sg], bf16, tag="tbsl")
            nc.vector.tensor_copy(out=tb_sl[:], in_=t_sl[:])

            s_em = sb.tile([P, 4 * cap], f32, tag="sem")
            zpall = ps.tile([P, Sg // 2], bf16, space="PSUM", tag="zpall", bufs=1)

            for j0, w in _slabs(Sg):
                xb = ps2.tile([EMB, SLAB], f32, space="PSUM", tag="slabp")
                nc.tensor.matmul(out=xb[:, :w], lhsT=cb("ones1"),
                                 rhs=x_sl[:, j0:j0 + w], start=True, stop=True)
                e1 = sb.tile([EMB, SLAB], bf16, tag="ve1")
                nc.scalar.activation(out=e1[:, :w], in_=xb[:, :w],
                                     func=Relu, scale=cf("vW1"), bias=cf("vB1"))
                e2p = ps2.tile([EMB, SLAB], f32, space="PSUM", tag="slabp")
                nc.tensor.matmul(out=e2p[:, :w], lhsT=cb("vW2T"), rhs=e1[:, :w],
                                 start=True, stop=True)
                e2 = sb.tile([EMB, SLAB], bf16, tag="ve2")
                nc.scalar.activation(out=e2[:, :w], in_=e2p[:, :w], func=Relu)
                ap_ = ps2.tile([EMB, SLAB], f32, space="PSUM", tag="slaba")
                nc.tensor.matmul(out=ap_[:, :w], lhsT=cb("vWl"), rhs=e2[:, :w],
                                 start=True, stop=False)
                y0 = sb.tile([EMB, SLAB], bf16, tag="vy0")
                nc.vector.tensor_copy(out=y0[:, :w], in_=ap_[:, :w])
                sq = sb.tile([EMB, SLAB], bf16, tag="vsq")
                nc.vector.tensor_tensor(out=sq[:, :w], in0=y0[:, :w],
                                        in1=y0[:, :w], op=MULT)
                stp = ps.tile([3, SLAB], f32, space="PSUM", tag="smallp")
                nc.tensor.matmul(out=stp[0:3, :w], lhsT=cb("vstat3"),
                                 rhs=y0[:, :w], start=True, stop=True)
                stp2 = ps.tile([1, SLAB], f32, space="PSUM", tag="smallp")
                nc.tensor.matmul(out=stp2[0:1, :w], lhsT=cb("ones64d"),
                                 rhs=sq[:, :w], start=True, stop=True)
                sts = sb.tile([3, SLAB], f32, tag="vsts")
                nc.vector.tensor_copy(out=sts[0:3, :w], in_=stp[0:3, :w])
                sts2 = sb.tile([1, SLAB], f32, tag="vsts2")
                nc.vector.tensor_copy(out=sts2[0:1, :w], in_=stp2[0:1, :w])
                # mu row (s1) -> bf16 for rank-1
                mub = sb.tile([1, SLAB], bf16, tag="vmub")
                nc.vector.tensor_copy(out=mub[:, :w], in_=sts[0:1, :w])
                nc.tensor.matmul(out=ap_[:, :w], lhsT=cb("negones1"),
                                 rhs=mub[:, :w], start=False, stop=False)
                nc.tensor.matmul(out=ap_[:, :w], lhsT=cb("what1_vc"),
                                 rhs=tb_sl[:, j0:j0 + w], start=False, stop=True)
                ysb = sb.tile([EMB, SLAB], bf16, tag="vysb")
                nc.scalar.activation(out=ysb[:, :w], in_=ap_[:, :w],
                                     func=Identity, bias=cf("bhat_vc"))
                for cc in range(w // P):
                    c = (j0 + cc * P) // P
                    zslice = bass.AP(zpall.tensor, zpall.offset + c * EMB * 2,
                                     [(zpall.ap[0][0], P), (1, EMB)]) if False \
                        else zpall[:, c * EMB:(c + 1) * EMB]
                    nc.tensor.transpose(out=zslice,
                                        in_=ysb[:, cc * P:(cc + 1) * P],
                                        identity=cb("identEb"))
                # stats transposes per chunk
                for cc in range(w // P):
                    c = (j0 + cc * P) // P
                    tp = ps.tile([P, 4], f32, space="PSUM", tag="smallp")
                    nc.tensor.transpose(out=tp[:, 0:3],
                                        in_=sts[0:3, cc * P:(cc + 1) * P],
                                        identity=cf("ident4")[0:3, 0:3])
                    nc.tensor.transpose(out=tp[:, 3:4],
                                        in_=sts2[0:1, cc * P:(cc + 1) * P],
                                        identity=cf("ident4")[0:1, 0:1])
                    nc.vector.tensor_copy(out=s_em[:, c * 4:(c + 1) * 4],
                                          in_=tp[:])
            # alpha pipe [P, cap]
            sv = s_em[:].rearrange("p (c k) -> p c k", c=cap, k=4)
            al = _alpha(nc, sb, cf, sv, t_eml, m_eml, cap,
                        WB["q2_vc"], WB["c_bb_vc"], WB["c_bw_vc"], "va")
            # relu + alpha + aggregate
            zeall = sb.tile([P, Sg // 2], bf16, tag="zeall")
            nc.scalar.activation(out=zeall[:], in_=zpall[:], func=Relu)
            zs = sb.tile([P, Sg // 2], bf16, tag="zsall")
            alv = al[:, :, None].broadcast_to([P, cap, EMB])
            zev = zeall[:].rearrange("p (c f) -> p c f", c=cap, f=EMB)
            zsv = zs[:].rearrange("p (c f) -> p c f", c=cap, f=EMB)
            nc.vector.tensor_tensor(out=zsv, in0=zev, in1=alv, op=MULT)
            aggp = ps.tile([P, EMB], f32, space="PSUM", tag="aggp", bufs=1)
            for c in range(cap):
                nc.tensor.matmul(out=aggp[:], lhsT=cb("ident128"),
                                 rhs=zs[:, c * EMB:(c + 1) * EMB],
                                 start=(c == 0), stop=(c == cap - 1))
            # ---- node MLP + A2 table for this tile ----
            ag_sb = sb.tile([P, EMB], bf16, tag="nagsb")
            nc.vector.tensor_copy(out=ag_sb[:], in_=aggp[:])
            agT_p = ps.tile([EMB, P], bf16, space="PSUM", tag="nodep", bufs=1)
            nc.tensor.transpose(out=agT_p[:], in_=ag_sb[:],
                                identity=cb("ident128"))
            agT = sb.tile([EMB, P], bf16, tag="nagt")
            nc.vector.tensor_copy(out=agT[:], in_=agT_p[:])
            right = sb.tile([EMB, P], bf16, tag="nright")
            nc.sync.dma_start(out=right[:], in_=cemb_tab[:, r * P:(r + 1) * P])
            up = ps.tile([EMB, P], f32, space="PSUM", tag="nodep", bufs=1)
            nc.tensor.matmul(out=up[:], lhsT=cb("M1_vc"), rhs=agT[:],
                             start=True, stop=False)
            nc.tensor.matmul(out=up[:], lhsT=cb("M2_vc"), rhs=right[:],
                             start=False, stop=True)
            u = sb.tile([EMB, P], bf16, tag="nu")
            nc.scalar.activation(out=u[:], in_=up[:], func=Relu)
            cnp = ps.tile([EMB, P], f32, space="PSUM", tag="nodep", bufs=1)
            nc.tensor.matmul(out=cnp[:], lhsT=cb("Wo2_vc"), rhs=u[:],
                             start=True, stop=True)
            cn = sb.tile([EMB, P], bf16, tag="ncn")
            nc.vector.tensor_copy(out=cn[:], in_=cnp[:])
            a2p = ps.tile([EMB + 1, P], f32, space="PSUM", tag="nodep", bufs=1)
            nc.tensor.matmul(out=a2p[:], lhsT=cb("cvWlmu"), rhs=cn[:],
                             start=True, stop=False)
            mu2r = sb.tile([1, P], bf16, tag="nmu2")
            nc.vector.tensor_copy(out=mu2r[:], in_=a2p[EMB:EMB + 1, :])
            nc.tensor.matmul(out=a2p[0:EMB, :], lhsT=cb("negones1"),
                             rhs=mu2r[:], start=False, stop=True,
                             skip_group_check=True)
            u2 = sb.tile([EMB, P], f32, tag="nu2")
            nc.scalar.activation(out=u2[:], in_=a2p[0:EMB, :], func=Identity,
                                 bias=cf("bhat_cv"))
            u2b = sb.tile([EMB, P], bf16, tag="nu2b")
            nc.vector.tensor_copy(out=u2b[:], in_=u2[:])
            u2sq = sb.tile([EMB, P], bf16, tag="nu2sq")
            nc.vector.tensor_tensor(out=u2sq[:], in0=u2b[:], in1=u2b[:], op=MULT)
            qp = ps.tile([1, P], f32, space="PSUM", tag="smallp")
            nc.tensor.matmul(out=qp[0:1, :], lhsT=cb("ones64d"), rhs=u2sq[:],
                             start=True, stop=True)
            qs0 = sb.tile([1, P], f32, tag="nqs0")
            nc.vector.tensor_copy(out=qs0[:], in_=qp[0:1, :])
            qp2 = ps.tile([1, P], f32, space="PSUM", tag="smallp")
            nc.tensor.matmul(out=qp2[0:1, :], lhsT=cb("what2d"), rhs=u2b[:],
                             start=True, stop=True)
            qs1 = sb.tile([1, P], f32, tag="nqs1")
            nc.vector.tensor_copy(out=qs1[:], in_=qp2[0:1, :])
            nc.sync.dma_start(out=a2_tab[0:EMB, r * P:(r + 1) * P], in_=u2[:])
            nc.sync.dma_start(out=a2_tab[EMB:EMB + 1, r * P:(r + 1) * P],
                              in_=qs0[:])
            nc.sync.dma_start(out=a2_tab[EMB + 1:EMB + 2, r * P:(r + 1) * P],
                              in_=qs1[:])
            soff += Sg

    nc.compile()
    return nc


def _alpha(nc, sb, cf, sv, t_eml, m_eml, cap, q2, c_bb, c_bw, tg):
    """var = (s2 - s1^2 + s3 + c_bb) + t*(s4 + 2*c_bw) + t^2*q2; al = m/sqrt(var+eps)
    sv: [P, cap, 4] strided stats view (s3, s4, s1, s2)."""
    s3c = sb.tile([P, cap], f32, tag=f"{tg}3")
    nc.vector.tensor_scalar(out=s3c[:], in0=sv[:, :, 1], scalar1=float(c_bb),
                            scalar2=None, op0=ADD)
    s4c = sb.tile([P, cap], f32, tag=f"{tg}4")
    nc.vector.tensor_scalar(out=s4c[:], in0=sv[:, :, 2], scalar1=float(2 * c_bw),
                            scalar2=None, op0=ADD)
    v1 = sb.tile([P, cap], f32, tag=f"{tg}v1")
    nc.vector.scalar_tensor_tensor(out=v1[:], in0=t_eml[:], scalar=float(q2),
                                   in1=s4c[:], op0=MULT, op1=ADD)
    v2 = sb.tile([P, cap], f32, tag=f"{tg}v2")
    nc.vector.tensor_tensor(out=v2[:], in0=v1[:], in1=t_eml[:], op=MULT)
    v3 = sb.tile([P, cap], f32, tag=f"{tg}v3")
    nc.vector.tensor_tensor(out=v3[:], in0=sv[:, :, 0], in1=sv[:, :, 0], op=MULT)
    v4 = sb.tile([P, cap], f32, tag=f"{tg}v4")
    nc.vector.tensor_tensor(out=v4[:], in0=sv[:, :, 3], in1=v3[:], op=SUB)
    v5 = sb.tile([P, cap], f32, tag=f"{tg}v5")
    nc.vector.tensor_tensor(out=v5[:], in0=v4[:], in1=s3c[:], op=ADD)
    var = sb.tile([P, cap], f32, tag=f"{tg}vr")
    nc.vector.tensor_tensor(out=var[:], in0=v5[:], in1=v2[:], op=ADD)
    sd = sb.tile([P, cap], f32, tag=f"{tg}sd")
    nc.scalar.activation(out=sd[:], in_=var[:], func=Sqrt, bias=cf("eps"),
                         scale=1.0)
    rs = sb.tile([P, cap], f32, tag=f"{tg}rs")
    nc.vector.reciprocal(out=rs[:], in_=sd[:])
    al = sb.tile([P, cap], f32, tag=f"{tg}al")
    nc.vector.tensor_tensor(out=al[:], in0=rs[:], in1=m_eml[:], op=MULT)
    alb = sb.tile([P, cap], bf16, tag=f"{tg}ab")
    nc.vector.tensor_copy(out=alb[:], in_=al[:])
    return alb


def build_l2(shapes, WB):
    S = shapes["S_cv"]
    caps = shapes["caps_cv"]
    n_rounds = len(caps)
    n_vt = shapes["n_vt"]
    CF, CB = shapes["CF"], shapes["CB"]

    nc = bacc.Bacc("TRN2", target_bir_lowering=False, debug=False,
                   num_devices=NCORES)
    u2_fm = nc.dram_tensor("u2_fm", [EMB, S], bf16, kind="ExternalInput").ap()
    t_fm = nc.dram_tensor("t2_fm", [1, S], bf16, kind="ExternalInput").ap()
    t_em = nc.dram_tensor("t2_em", [P, S // P], f32, kind="ExternalInput").ap()
    m_em = nc.dram_tensor("m2_em", [P, S // P], f32, kind="ExternalInput").ap()
    q0_em = nc.dram_tensor("q0_em", [P, S // P], f32, kind="ExternalInput").ap()
    q1_em = nc.dram_tensor("q1_em", [P, S // P], f32, kind="ExternalInput").ap()
    vemb_tab = nc.dram_tensor("vemb_tab", [EMB, n_vt], bf16,
                              kind="ExternalInput").ap()
    head_out = nc.dram_tensor("head_out", [1, 1], f32, kind="ExternalOutput").ap()

    with tile.TileContext(nc) as tc, ExitStack() as ctx0:
        sb = ctx0.enter_context(tc.tile_pool(name="sb", bufs=3))
        sc = ctx0.enter_context(tc.tile_pool(name="sc", bufs=1))
        ps = ctx0.enter_context(tc.tile_pool(name="ps", bufs=1, space="PSUM"))
        ps2 = ctx0.enter_context(tc.tile_pool(name="ps2", bufs=1, space="PSUM"))

        cw, cwb = _load_consts(nc, sc, shapes["cf_shape"], shapes["cb_shape"])

        def cf(name):
            c0, w, h = CF[name]
            return cw[0:h, c0:c0 + w]

        def cb(name):
            c0, w, h = CB[name]
            return cwb[0:h, c0:c0 + w]

        headp = ps.tile([1, P], f32, space="PSUM", tag="headp", bufs=1)
        soff = 0
        for r in range(n_rounds):
            cap = int(caps[r])
            Sg = cap * P
            co = soff // P
            t_eml = sb.tile([P, cap], f32, tag="teml")
            nc.sync.dma_start(out=t_eml[:], in_=t_em[:, co:co + cap])
            m_eml = sb.tile([P, cap], f32, tag="meml")
            nc.sync.dma_start(out=m_eml[:], in_=m_em[:, co:co + cap])
            q0l = sb.tile([P, cap], f32, tag="q0l")
            nc.sync.dma_start(out=q0l[:], in_=q0_em[:, co:co + cap])
            q1l = sb.tile([P, cap], f32, tag="q1l")
            nc.sync.dma_start(out=q1l[:], in_=q1_em[:, co:co + cap])
            t_sl = sb.tile([1, Sg], bf16, tag="tsl")
            nc.gpsimd.dma_start(out=t_sl[:], in_=t_fm[:, soff:soff + Sg])
            u2l = sb.tile([EMB, Sg], bf16, tag="u2l")
            nc.sync.dma_start(out=u2l[:], in_=u2_fm[:, soff:soff + Sg])

            # y2 = u2 + t*what2 (rank-1 psum + add, per slab)
            y2 = sb.tile([EMB, Sg], bf16, tag="y2")
            for j0, w in _slabs(Sg):
                twp = ps2.tile([EMB, SLAB], f32, space="PSUM", tag="slabp")
                nc.tensor.matmul(out=twp[:, :w], lhsT=cb("what1_cv"),
                                 rhs=t_sl[:, j0:j0 + w], start=True, stop=True)
                nc.vector.tensor_tensor(out=y2[:, j0:j0 + w],
                                        in0=u2l[:, j0:j0 + w],
                                        in1=twp[:, :w], op=ADD)

            zpall = ps.tile([P, Sg // 2], bf16, space="PSUM", tag="zpall", bufs=1)
            for c in range(cap):
                nc.tensor.transpose(out=zpall[:, c * EMB:(c + 1) * EMB],
                                    in_=y2[:, c * P:(c + 1) * P],
                                    identity=cb("identEb"))
            # alpha2: var = q0 + t*q1 + t^2 q2
            v1 = sb.tile([P, cap], f32, tag="cv1")
            nc.vector.scalar_tensor_tensor(out=v1[:], in0=t_eml[:],
                                           scalar=float(WB["q2_cv"]),
                                           in1=q1l[:], op0=MULT, op1=ADD)
            v2 = sb.tile([P, cap], f32, tag="cv2")
            nc.vector.tensor_tensor(out=v2[:], in0=v1[:], in1=t_eml[:], op=MULT)
            var = sb.tile([P, cap], f32, tag="cvr")
            nc.vector.tensor_tensor(out=var[:], in0=v2[:], in1=q0l[:], op=ADD)
            sd = sb.tile([P, cap], f32, tag="csd")
            nc.scalar.activation(out=sd[:], in_=var[:], func=Sqrt,
                                 bias=cf("eps"), scale=1.0)
            rs = sb.tile([P, cap], f32, tag="crs")
            nc.vector.reciprocal(out=rs[:], in_=sd[:])
            al = sb.tile([P, cap], f32, tag="cal")
            nc.vector.tensor_tensor(out=al[:], in0=rs[:], in1=m_eml[:], op=MULT)
            alb = sb.tile([P, cap], bf16, tag="cab")
            nc.vector.tensor_copy(out=alb[:], in_=al[:])

            zeall = sb.tile([P, Sg // 2], bf16, tag="zeall")
            nc.scalar.activation(out=zeall[:], in_=zpall[:], func=Relu)
            zs = sb.tile([P, Sg // 2], bf16, tag="zsall")
            alv = alb[:, :, None].broadcast_to([P, cap, EMB])
            zev = zeall[:].rearrange("p (c f) -> p c f", c=cap, f=EMB)
            zsv = zs[:].rearrange("p (c f) -> p c f", c=cap, f=EMB)
            nc.vector.tensor_tensor(out=zsv, in0=zev, in1=alv, op=MULT)
            aggp = ps.tile([P, EMB], f32, space="PSUM", tag="aggp", bufs=2)
            for c in range(cap):
                nc.tensor.matmul(out=aggp[:], lhsT=cb("ident128"),
                                 rhs=zs[:, c * EMB:(c + 1) * EMB],
                                 start=(c == 0), stop=(c == cap - 1))
            # node MLP + head
            ag_sb = sb.tile([P, EMB], bf16, tag="nagsb")
            nc.vector.tensor_copy(out=ag_sb[:], in_=aggp[:])
            agT_p = ps.tile([EMB, P], bf16, space="PSUM", tag="nodep", bufs=2)
            nc.tensor.transpose(out=agT_p[:], in_=ag_sb[:],
                                identity=cb("ident128"))
            agT = sb.tile([EMB, P], bf16, tag="nagt")
            nc.vector.tensor_copy(out=agT[:], in_=agT_p[:])
            right = sb.tile([EMB, P], bf16, tag="nright")
            nc.sync.dma_start(out=right[:], in_=vemb_tab[:, r * P:(r + 1) * P])
            up = ps.tile([EMB, P], f32, space="PSUM", tag="nodep", bufs=2)
            nc.tensor.matmul(out=up[:], lhsT=cb("M1_cv"), rhs=agT[:],
                             start=True, stop=False)
            nc.tensor.matmul(out=up[:], lhsT=cb("M2_cv"), rhs=right[:],
                             start=False, stop=True)
            u = sb.tile([EMB, P], bf16, tag="nu")
            nc.scalar.activation(out=u[:], in_=up[:], func=Relu)
            vnp = ps.tile([EMB, P], f32, space="PSUM", tag="nodep", bufs=2)
            nc.tensor.matmul(out=vnp[:], lhsT=cb("Wo2_cv"), rhs=u[:],
                             start=True, stop=True)
            vn = sb.tile([EMB, P], bf16, tag="nvn")
            nc.vector.tensor_copy(out=vn[:], in_=vnp[:])
            pp = ps.tile([EMB, P], f32, space="PSUM", tag="nodep", bufs=2)
            nc.tensor.matmul(out=pp[:], lhsT=cb("Wp1"), rhs=vn[:],
                             start=True, stop=True)
            pr = sb.tile([EMB, P], bf16, tag="npr")
            nc.scalar.activation(out=pr[:], in_=pp[:], func=Relu)
            nc.tensor.matmul(out=headp[:], lhsT=cb("Wp2col"), rhs=pr[:],
                             start=(r == 0), stop=(r == n_rounds - 1))
            soff += Sg

        hsb = sb.tile([1, P], f32, tag="hsb")
        nc.vector.tensor_copy(out=hsb[:], in_=headp[:])
        hred = sb.tile([1, 1], f32, tag="hred")
        nc.vector.tensor_reduce(out=hred[:], in_=hsb[:],
                                axis=mybir.AxisListType.X, op=ADD)
        nc.sync.dma_start(out=head_out[:], in_=hred[:])

    nc.compile()
    return nc


# ---------------------------------------------------------------------------
# host orchestration
# ---------------------------------------------------------------------------
def kernel(constraint_features, edge_indices, edge_features, variable_features,
           cons_emb, var_emb, conv_vc, conv_cv, out_mlp):
    from concourse.bass_utils import run_bass_kernel_spmd

    NC_ = constraint_features.shape[0]
    NV_ = variable_features.shape[0]
    ci = np.asarray(edge_indices[0], np.int64)
    vi = np.asarray(edge_indices[1], np.int64)
    xc = np.asarray(constraint_features, np.float32).reshape(-1)
    xv = np.asarray(variable_features, np.float32).reshape(-1)
    ef = np.asarray(edge_features, np.float32).reshape(-1)

    cW1, cW2 = [np.asarray(a, np.float32) for a in cons_emb]
    vW1, vW2 = [np.asarray(a, np.float32) for a in var_emb]
    (vc_Wl, vc_bl, vc_We, vc_be, vc_g, vc_b, vc_Wf, vc_Wo1, vc_Wo2) = \
        [np.asarray(a, np.float32) for a in conv_vc]
    (cv_Wl, cv_bl, cv_We, cv_be, cv_g, cv_b, cv_Wf, cv_Wo1, cv_Wo2) = \
        [np.asarray(a, np.float32) for a in conv_cv]
    Wp1, Wp2 = [np.asarray(a, np.float32) for a in out_mlp]
    assert np.all(vc_g == 1) and np.all(vc_b == 0)
    assert np.all(cv_g == 1) and np.all(cv_b == 0)

    w_vc = vc_We.reshape(-1)
    what_vc = w_vc - w_vc.mean()
    b_vc = vc_bl + vc_be
    bhat_vc = b_vc - b_vc.mean()
    w_cv = cv_We.reshape(-1)
    what_cv = w_cv - w_cv.mean()
    b_cv = cv_bl + cv_be
    bhat_cv = b_cv - b_cv.mean()
    WB = {
        "q2_vc": float((what_vc ** 2).mean()),
        "c_bb_vc": float((bhat_vc ** 2).mean()),
        "c_bw_vc": float((bhat_vc * what_vc).mean()),
        "q2_cv": float((what_cv ** 2).mean()),
    }

    plan_vc = ConvPlan(ci, NC_)
    plan_cv = ConvPlan(vi, NV_)

    # ---- constants ----
    cfp = ConstPack(P, f32)
    cfp.add("vW1", vW1.reshape(-1))          # [64]
    cfp.add("vB1", np.zeros(EMB))            # no bias in embed lin1
    cfp.add("cW1", cW1.reshape(-1))
    cfp.add("cB1", np.zeros(EMB))
    cfp.add("bhat_vc", bhat_vc)
    cfp.add("bhat_cv", bhat_cv)
    cfp.add("eps", np.full(P, LN_EPS))
    cfp.add("what2col", what_cv)
    cfp.add("ident4", np.eye(4))
    cf_blob = cfp.blob()

    cbp = ConstPack(P, bf16)
    cbp.add("vW2T", vW2)                     # lhsT [in,out] as stored
    cbp.add("cW2T", cW2)
    cbp.add("vWl", vc_Wl)
    cbp.add("vstat3", np.stack([np.ones(EMB) / EMB, 2 * bhat_vc / EMB,
                                2 * what_vc / EMB], axis=1))
    cbp.add("ones64d", np.ones(EMB)[:, None] / EMB)
    cbp.add("negones1", -np.ones((1, EMB)))
    cbp.add("ones1", np.ones((1, EMB)))
    cbp.add("what1_vc", what_vc[None, :])
    cbp.add("what1_cv", what_cv[None, :])
    cbp.add("identEb", np.eye(EMB))
    cbp.add("ident128", np.eye(P))
    cbp.add("M1_vc", vc_Wf @ vc_Wo1[:EMB])
    cbp.add("M2_vc", vc_Wo1[EMB:])
    cbp.add("Wo2_vc", vc_Wo2)
    cbp.add("cvWlmu", np.concatenate([cv_Wl, cv_Wl.mean(axis=1, keepdims=True)],
                                     axis=1))
    cbp.add("what2d", (2 * what_cv / EMB)[:, None])
    cbp.add("M1_cv", cv_Wf @ cv_Wo1[:EMB])
    cbp.add("M2_cv", cv_Wo1[EMB:])
    cbp.add("Wo2_cv", cv_Wo2)
    cbp.add("Wp1", Wp1)
    cbp.add("Wp2col", Wp2.reshape(EMB, 1))
    cb_blob = cbp.blob()

    # ---- streams L1 (conv v->c) ----
    x_slot = plan_vc.scatter(xv[vi])
    t_slot = plan_vc.scatter(ef)
    mask_ = np.zeros(len(ef))
    m_slot = plan_vc.scatter(np.ones(len(ef)))
    n_ct = plan_vc.n_rounds * P
    n_vt = plan_cv.n_rounds * P

    xc_perm = np.zeros((NCORES, n_ct), np.float32)
    mc_perm = np.zeros((NCORES, n_ct), np.float32)
    xv_perm = np.zeros((NCORES, n_vt), np.float32)
    mv_perm = np.zeros((NCORES, n_vt), np.float32)
    crows, vrows = [], []
    for c in range(NCORES):
        cr = plan_vc.target_rows(c)
        vr = plan_cv.target_rows(c)
        crows.append(cr)
        vrows.append(vr)
        vc_valid = cr >= 0
        xc_perm[c, vc_valid] = xc[cr[vc_valid]]
        mc_perm[c, vc_valid] = 1.0
        vv = vr >= 0
        xv_perm[c, vv] = xv[vr[vv]]
        mv_perm[c, vv] = 1.0

    shapes = {
        "S_vc": plan_vc.total_slots, "caps_vc": plan_vc.caps,
        "S_cv": plan_cv.total_slots, "caps_cv": plan_cv.caps,
        "n_ct": n_ct, "n_vt": n_vt,
        "CF": cfp.off, "CB": cbp.off,
        "cf_shape": cf_blob.shape, "cb_shape": cb_blob.shape,
    }

    nc1 = build_l1(shapes, WB)
    in_maps1 = []
    for c in range(NCORES):
        in_maps1.append({
            "consts_f": cf_blob, "consts_b": cb_blob,
            "x_fm": x_slot[c][None, :].astype(bf), "t_fm": t_slot[c][None, :],
            "t_em": t_slot[c].reshape(-1, P).T.copy(),
            "m_em": m_slot[c].reshape(-1, P).T.copy(),
            "xc_fm": xc_perm[c][None, :].astype(bf), "mc_fm": mc_perm[c][None, :].astype(bf),
            "xv_fm": xv_perm[c][None, :].astype(bf), "mv_fm": mv_perm[c][None, :].astype(bf),
        })
    res1 = run_bass_kernel_spmd(nc1, in_maps1, list(range(NCORES)))

    # assemble tables: a2 rows per constraint id; vemb per core
    a2_full = np.zeros((EMB + 2, NC_), np.float32)
    for c in range(NCORES):
        tab = np.asarray(res1.results[c]["a2_tab"], np.float32)
        cr = crows[c]
        valid = cr >= 0
        a2_full[:, cr[valid]] = tab[:, valid]
    vembs = [np.asarray(res1.results[c]["vemb_tab"]) for c in range(NCORES)]

    # ---- streams L2 (conv c->v): expand a2 table per slot ----
    src_slot = plan_cv.scatter(ci.astype(np.float64)).astype(np.int64)
    # note: scatter fills float; redo exactly:
    src_slot = np.zeros((NCORES, plan_cv.total_slots), np.int64)
    src_slot[plan_cv.edge_core, plan_cv.edge_pos] = ci
    t2_slot = plan_cv.scatter(ef)
    m2_slot = plan_cv.scatter(np.ones(len(ef)))

    nc2 = build_l2(shapes, WB)
    in_maps2 = []
    for c in range(NCORES):
        rows = a2_full[:, src_slot[c]]                  # [66, S_cv]
        u2 = rows[0:EMB].astype(bf)
        q0 = rows[EMB].astype(np.float32)
        q1 = rows[EMB + 1].astype(np.float32)
        in_maps2.append({
            "consts_f": cf_blob, "consts_b": cb_blob,
            "u2_fm": u2,
            "t2_fm": t2_slot[c][None, :].astype(bf),
            "t2_em": t2_slot[c].reshape(-1, P).T.copy(),
            "m2_em": m2_slot[c].reshape(-1, P).T.copy(),
            "q0_em": q0.reshape(-1, P).T.copy(),
            "q1_em": q1.reshape(-1, P).T.copy(),
            "vemb_tab": vembs[c],
        })
    res2 = run_bass_kernel_spmd(nc2, in_maps2, list(range(NCORES)))

    total = sum(float(np.asarray(res2.results[c]["head_out"])[0, 0])
                for c in range(NCORES))
    out = 1.0 / (1.0 + np.exp(-(total / NV_)))
    return np.array([out], np.float32)
